# revision 21
# baseline (speedup 1.0000x reference)
"""Multi-head attention (B=4, N=2048, C=768, H=12) on 8 trn2 NeuronCores.

Sharding: core c handles batch b = c//2 and query rows [ (c%2)*1024, +1024 ).

Per-core engine plan (v2):
  PE     : QKV/proj linears, QK^T (2 heads packed on row groups), PV with the
           two q-chunks packed on column groups (full 128-wide array), ones
           matmuls accumulating softmax denominators on psum rows 0/32/64/96,
           reciprocal-broadcast matmuls.
  ScalarE: exp for even heads (table exp), denominator reciprocal via
           ln -> exp(-x), psum->sbuf drains of the linears.
  VectorE: exp for odd heads via Schraudolph int16 bit-trick (one
           tensor_scalar: i16 = st*A + B, bitcast bf16), attention drains,
           normalize muls.
  DMA    : host-side bf16 inputs; sbuf->sbuf shuffles assemble the normalized
           attention output into projection layout during attention.
  Emission interleaves V (pair 0) and next pair's Q/K (pairs 1-4) into the
  attention loop so the PE never idles and HAM stays at full clock.

PSUM budget (8 banks): st 2x[128,1024] = 4, t 2x[128,512] = 2, den 1, rb 1.
"""

import os
import sys

import numpy as np
import ml_dtypes

sys.path.insert(0, "/opt/trn_rl_repo")

import concourse.bass as bass
from concourse import bacc
import concourse.mybir as mybir
from concourse.tile import TileContext
from concourse.bass_utils import run_bass_kernel_spmd
from concourse.dma_utils import dma_copy

P = 128
C = 768
NK = 2048
NQ = 1024
H = 12
DH = 64
CT = C // P          # 6 c-tiles (contraction tiles for the linears)
KT = NK // P         # 16 key tiles
QCH = 512            # q-chunk (max psum bank free dim for fp32)
SCALE = DH ** -0.5
F32 = mybir.dt.float32
BF16 = mybir.dt.bfloat16
I16 = mybir.dt.int16
EXP = mybir.ActivationFunctionType.Exp
LN = mybir.ActivationFunctionType.Ln

LOG2E = 1.4426950408889634
A16 = 128.0 * LOG2E * SCALE          # fold the 1/8 attention scale
B16 = 127.0 * 128.0 - 0.043677448 * 128.0

LAST_RESULT = None
_PROG = None


def _build_program() -> bass.Bass:
    nc = bacc.Bacc(None, target_bir_lowering=False)

    # host supplies bf16 (halves DMA bytes; matches baseline numerics)
    wqt = nc.dram_tensor("wqt", [C, C], BF16, kind="ExternalInput")
    wkt = nc.dram_tensor("wkt", [C, C], BF16, kind="ExternalInput")
    xq = nc.dram_tensor("xq", [C, NQ], BF16, kind="ExternalInput")
    xt = nc.dram_tensor("xt", [C, NK], BF16, kind="ExternalInput")
    wvt = nc.dram_tensor("wvt", [C, C], BF16, kind="ExternalInput")
    wpt = nc.dram_tensor("wpt", [C, C], BF16, kind="ExternalInput")
    bp = nc.dram_tensor("bp", [1, C], BF16, kind="ExternalInput")
    y = nc.dram_tensor("y", [NQ, C], F32, kind="ExternalOutput")

    with TileContext(nc) as tc:
        with (
            tc.tile_pool(name="persist", bufs=1) as persist,
            tc.tile_pool(name="pt0p", bufs=2) as pt0p,
            tc.tile_pool(name="pt1p", bufs=2) as pt1p,
            tc.tile_pool(name="small", bufs=2) as small,
            tc.tile_pool(name="ysb", bufs=2) as ysb,
            tc.tile_pool(name="ps_st", bufs=2, space="PSUM") as ps_st,
            tc.tile_pool(name="ps_t", bufs=2, space="PSUM") as ps_t,
            tc.tile_pool(name="ps_den", bufs=1, space="PSUM") as ps_den,
            tc.tile_pool(name="ps_rb", bufs=1, space="PSUM") as ps_rb,
        ):
            # ---- load inputs (DMA order = dependency order) ----
            def load(cols, tag):
                return [
                    persist.tile([P, cols], BF16, tag=f"{tag}{i}", name=f"{tag}{i}")
                    for i in range(CT)
                ]

            def dma_tiles(tiles, dram, cols, col0=0):
                for i, t in enumerate(tiles):
                    dma_copy(
                        nc.gpsimd,
                        t[:, col0:col0 + cols],
                        dram[i * P:(i + 1) * P, col0:col0 + cols],
                    )

            wqb = load(C, "wqb")
            wkb = load(C, "wkb")
            xqb = load(NQ, "xqb")
            xtb = load(NK, "xtb")
            wvb = load(C, "wvb")
            wpb = load(C, "wpb")

            def dma_one(t, dram, i, cols, col0=0):
                dma_copy(
                    nc.gpsimd,
                    t[:, col0:col0 + cols],
                    dram[i * P:(i + 1) * P, col0:col0 + cols],
                )

            for i in range(CT):   # Q path first, tile-interleaved
                dma_one(wqb[i], wqt, i, C)
                dma_one(xqb[i], xq, i, NQ)
            for i in range(CT):   # K path
                dma_one(wkb[i], wkt, i, C)
                dma_one(xtb[i], xt, i, NQ, col0=0)
            dma_tiles(xtb, xt, NQ, col0=NQ)     # second half of keys
            dma_tiles(wvb, wvt, C)
            dma_tiles(wpb, wpt, C)

            bpb = persist.tile([1, C], BF16, tag="bpb")
            dma_copy(nc.gpsimd, bpb[:, :], bp[:, :])

            ones = persist.tile([P, P], BF16, tag="ones")
            nc.gpsimd.memset(ones[:, :], 1.0)

            qtb = [persist.tile([P, NQ], BF16, tag=f"qt{i}", name=f"qt{i}") for i in range(CT)]
            ktb = [persist.tile([P, NK], BF16, tag=f"kt{i}", name=f"kt{i}") for i in range(CT)]
            vtb = [persist.tile([P, C], BF16, tag=f"v{i}", name=f"v{i}") for i in range(KT)]
            otb = [persist.tile([P, NQ], BF16, tag=f"ot{i}", name=f"ot{i}") for i in range(CT)]

            def emit_q_chunk(hp, j, pool):
                ps = pool.tile([P, QCH], F32, tag="st")
                for k in range(CT):
                    nc.tensor.matmul(
                        ps[:, :],
                        lhsT=wqb[k][:, hp * P:(hp + 1) * P],
                        rhs=xqb[k][:, j * QCH:(j + 1) * QCH],
                        start=(k == 0), stop=(k == CT - 1),
                    )
                nc.scalar.copy(qtb[hp][:, j * QCH:(j + 1) * QCH], ps[:, :])

            def emit_q(hp, pool):
                for j in range(2):
                    emit_q_chunk(hp, j, pool)

            def emit_k_chunk(hp, j, pool):
                ps = pool.tile([P, QCH], F32, tag="st")
                for k in range(CT):
                    nc.tensor.matmul(
                        ps[:, :],
                        lhsT=wkb[k][:, hp * P:(hp + 1) * P],
                        rhs=xtb[k][:, j * QCH:(j + 1) * QCH],
                        start=(k == 0), stop=(k == CT - 1),
                    )
                nc.scalar.copy(ktb[hp][:, j * QCH:(j + 1) * QCH], ps[:, :])

            def emit_v_chunk(i, ch, pool):
                # vtb[i] = x rows [i*128,+128) @ Wv.T -> [128, 768]
                c0, csz = (0, QCH) if ch == 0 else (QCH, C - QCH)
                ps = pool.tile([P, QCH], F32, tag="st")
                for k in range(CT):
                    nc.tensor.matmul(
                        ps[:, 0:csz],
                        lhsT=xtb[k][:, i * P:(i + 1) * P],
                        rhs=wvb[k][:, c0:c0 + csz],
                        start=(k == 0), stop=(k == CT - 1),
                    )
                nc.scalar.copy(vtb[i][:, c0:c0 + csz], ps[:, 0:csz])

            def emit_v(i, pool):
                for ch in range(2):
                    emit_v_chunk(i, ch, pool)

            # ---- initial linears: Q/K pairs 0,1 + V tile 0 (DMA-bound phase) ----
            emit_q(0, ps_st)
            for j in range(4):
                emit_k_chunk(0, j, ps_st)
            emit_v(0, ps_st)
            emit_q(1, ps_st)
            for j in range(4):
                emit_k_chunk(1, j, ps_st)

            ypart = [
                persist.tile([P, C], F32, tag=f"yp{i}", name=f"yp{i}")
                for i in range(NQ // P)
            ]

            def emit_proj_part(qi, ch):
                # first 4 head-pair contributions + bias for q-tile qi, chunk ch
                c0, csz = (0, QCH) if ch == 0 else (QCH, C - QCH)
                ps = ps_rb.tile([P, QCH], F32, tag="st", name="pp")
                for t in range(4):
                    nc.tensor.matmul(
                        ps[:, 0:csz],
                        lhsT=otb[t][:, qi * P:(qi + 1) * P],
                        rhs=wpb[t][:, c0:c0 + csz],
                        start=(t == 0), stop=False,
                    )
                nc.tensor.matmul(
                    ps[:, 0:csz],
                    lhsT=ones[0:1, 0:P],
                    rhs=bpb[0:1, c0:c0 + csz],
                    start=False, stop=True, tile_position=(0, 0),
                )
                if ch == 0:
                    nc.scalar.copy(ypart[qi][:, c0:c0 + csz], ps[:, 0:csz])
                else:
                    nc.vector.tensor_copy(ypart[qi][:, c0:c0 + csz], ps[:, 0:csz])

            def make_tail_deferred(hp, dsb, tmp0, tmp1):
                # reciprocal (chunked on DVE), broadcast, normalize, assemble —
                # sprinkled into the NEXT pair's k-tile slots
                state = {}

                def recip_chunk(c):
                    if "recb" not in state:
                        state["recb"] = small.tile(
                            [97, QCH], BF16, tag="recb", name="recb"
                        )
                    with nc.allow_low_precision(
                        reason="1/den as bf16 multiplier, matches baseline"
                    ):
                        nc.vector.reciprocal(
                            state["recb"][:, c * DH:(c + 1) * DH],
                            dsb[:, c * DH:(c + 1) * DH],
                        )

                def finish():
                    recb = state["recb"]
                    for h, tmp in ((0, tmp0), (1, tmp1)):
                        rb = ps_rb.tile([P, QCH], F32, tag="st", name="rb")
                        r0, r1 = (0, 32) if h == 0 else (64, 96)
                        nc.tensor.matmul(
                            rb[0:DH, :], lhsT=ones[r0:r0 + 1, 0:DH],
                            rhs=recb[r0:r0 + 1, :],
                            start=True, stop=True, tile_position=(r0, 0),
                        )
                        nc.tensor.matmul(
                            rb[DH:P, :], lhsT=ones[r1:r1 + 1, 0:DH],
                            rhs=recb[r1:r1 + 1, :],
                            start=True, stop=True, tile_position=(r1, 64),
                        )
                        osb = small.tile([P, QCH], BF16, tag="osb", name="osb")
                        nc.vector.tensor_mul(osb[:, :], tmp[:, :], rb[:, :])
                        nc.sync.dma_start(
                            out=otb[hp][h * DH:(h + 1) * DH, 0:QCH],
                            in_=osb[0:DH, :],
                        )
                        nc.sync.dma_start(
                            out=otb[hp][h * DH:(h + 1) * DH, QCH:NQ],
                            in_=osb[DH:P, :],
                        )

                fills = [lambda c=c: recip_chunk(c) for c in range(8)]
                fills.append(finish)
                return fills

            def make_fillers(hp, deferred):
                pe = []
                if hp == 0:
                    for i in range(1, KT):
                        for ch in range(2):
                            pe.append(
                                lambda i=i, ch=ch: emit_v_chunk(i, ch, ps_rb)
                            )
                elif hp < CT - 1:
                    nxt = hp + 1
                    for j in range(2):
                        pe.append(lambda j=j: emit_q_chunk(nxt, j, ps_rb))
                    for j in range(4):
                        pe.append(lambda j=j: emit_k_chunk(nxt, j, ps_rb))
                else:
                    for qi in range(NQ // P):
                        for ch in range(2):
                            pe.append(
                                lambda qi=qi, ch=ch: emit_proj_part(qi, ch)
                            )
                return list(deferred), pe

            deferred = []
            final_tail = None
            for hp in range(CT):
                h0, h1 = 2 * hp, 2 * hp + 1
                dfills, pfills = make_fillers(hp, deferred)
                npf = len(pfills)

                den = ps_den.tile([97, QCH], F32, tag="den")
                t_h0 = ps_t.tile([P, QCH], F32, tag="t")
                t_h1 = ps_t.tile([P, QCH], F32, tag="t")

                def av_den(i, pt0, pt1_i):
                    first, last = (i == 0), (i == KT - 1)
                    v0 = vtb[i][:, h0 * DH:(h0 + 1) * DH]
                    v1 = vtb[i][:, h1 * DH:(h1 + 1) * DH]

                    def pch(h, c):
                        sl = slice(c * QCH, (c + 1) * QCH)
                        return pt0[:, sl] if h == 0 else pt1_i[:, sl]

                    nc.tensor.matmul(
                        t_h0[0:DH, :], lhsT=v0, rhs=pch(0, 0),
                        start=first, stop=last, tile_position=(0, 0),
                    )
                    nc.tensor.matmul(
                        t_h0[DH:P, :], lhsT=v0, rhs=pch(0, 1),
                        start=first, stop=last, tile_position=(0, 64),
                    )
                    nc.tensor.matmul(
                        t_h1[0:DH, :], lhsT=v1, rhs=pch(1, 0),
                        start=first, stop=last, tile_position=(0, 0),
                    )
                    nc.tensor.matmul(
                        t_h1[DH:P, :], lhsT=v1, rhs=pch(1, 1),
                        start=first, stop=last, tile_position=(0, 64),
                    )
                    for pos, (h, c) in zip(
                        (0, 32, 64, 96), ((0, 0), (0, 1), (1, 0), (1, 1))
                    ):
                        nc.tensor.matmul(
                            den[pos:pos + 1, :],
                            lhsT=ones[:, pos:pos + 1],
                            rhs=pch(h, c),
                            start=first, stop=last, tile_position=(0, pos),
                        )

                pending = None   # (i, pt0, pt1_i) awaiting AV + den
                percap = 2 if npf > KT else 1
                for i in range(KT):
                    if i < len(dfills):
                        dfills[i]()
                    for f in range(
                        min(i * percap, npf), min((i + 1) * percap, npf)
                    ):
                        pfills[f]()
                    st1 = ps_st.tile([P, NQ], F32, tag="st")
                    st0 = ps_st.tile([P, NQ], F32, tag="st")
                    for j in range(2):
                        nc.tensor.matmul(
                            st1[:, j * QCH:(j + 1) * QCH],
                            lhsT=ktb[hp][DH:P, i * P:(i + 1) * P],
                            rhs=qtb[hp][DH:P, j * QCH:(j + 1) * QCH],
                            start=True, stop=True,
                            tile_position=(64, 0),
                        )
                        nc.tensor.matmul(
                            st0[:, j * QCH:(j + 1) * QCH],
                            lhsT=ktb[hp][0:DH, i * P:(i + 1) * P],
                            rhs=qtb[hp][0:DH, j * QCH:(j + 1) * QCH],
                            start=True, stop=True,
                            tile_position=(0, 0),
                        )
                    if pending is not None:
                        av_den(*pending)
                    # engine assignment alternates by k-tile parity so each
                    # S-buffer's reader alternates DVE (1.22us) / ScalarE
                    # (0.87us), shortening the ST -> exp -> ST buffer chain
                    pt1 = pt1p.tile([P, NQ], BF16, tag="pt1")
                    pt0 = pt0p.tile([P, NQ], BF16, tag="pt0")
                    dve_dst, dve_src, sc_dst, sc_src = (
                        (pt1, st1, pt0, st0) if i % 2 == 0
                        else (pt0, st0, pt1, st1)
                    )
                    nc.vector.tensor_scalar(
                        dve_dst[:, :].bitcast(I16), dve_src[:, :], A16, B16,
                        mybir.AluOpType.mult, mybir.AluOpType.add,
                    )
                    nc.scalar.activation(
                        sc_dst[:, :], sc_src[:, :], EXP, scale=SCALE
                    )
                    pending = (i, pt0, pt1)
                av_den(*pending)

                # immediate tail: release psum banks quickly (no table loads)
                dsb = small.tile([97, QCH], F32, tag="dsb")
                nc.scalar.copy(dsb[:, :], den[:, :])
                tmp0 = small.tile([P, QCH], F32, tag="tmp0")
                nc.scalar.copy(tmp0[:, :], t_h0[:, :])
                tmp1 = small.tile([P, QCH], F32, tag="tmp1")
                nc.vector.tensor_copy(tmp1[:, :], t_h1[:, :])
                deferred = make_tail_deferred(hp, dsb, tmp0, tmp1)

            # last pair's tail runs right here (nothing left to overlap with)
            for f in deferred:
                f()

            # ---- projection finish: add head-pairs 4,5 to the partials ----
            for qi in range(NQ // P):
                ps = ps_st.tile([P, NQ], F32, tag="st", name="pse")
                for t, last in ((4, False), (5, True)):
                    for (c0, csz) in ((0, QCH), (QCH, C - QCH)):
                        nc.tensor.matmul(
                            ps[:, c0:c0 + csz],
                            lhsT=otb[t][:, qi * P:(qi + 1) * P],
                            rhs=wpb[t][:, c0:c0 + csz],
                            start=(t == 4), stop=last,
                        )
                yt = ysb.tile([P, C], F32, tag="y")
                nc.vector.tensor_add(yt[:, :], ypart[qi][:, :], ps[:, 0:C])
                nc.sync.dma_start(out=y[qi * P:(qi + 1) * P, :], in_=yt[:, :])

    nc.compile()
    return nc


def _get_prog() -> bass.Bass:
    global _PROG
    if _PROG is None:
        _PROG = _build_program()
    return _PROG


def kernel(x, Wq, Wk, Wv, Wp, bp):
    global LAST_RESULT
    bf = ml_dtypes.bfloat16
    x = np.asarray(x, dtype=np.float32)
    wqt = np.ascontiguousarray(np.asarray(Wq, np.float32).T.astype(bf))
    wkt = np.ascontiguousarray(np.asarray(Wk, np.float32).T.astype(bf))
    wvt = np.ascontiguousarray(np.asarray(Wv, np.float32).T.astype(bf))
    wpt = np.ascontiguousarray(np.asarray(Wp, np.float32).T.astype(bf))
    bpv = np.ascontiguousarray(np.asarray(bp, np.float32).reshape(1, C).astype(bf))

    B, N, _ = x.shape
    in_maps = []
    for core in range(8):
        b, qh = core // 2, core % 2
        xtc = np.ascontiguousarray(x[b].T.astype(bf))
        xqc = np.ascontiguousarray(xtc[:, qh * NQ:(qh + 1) * NQ])
        in_maps.append({
            "xt": xtc, "xq": xqc,
            "wqt": wqt, "wkt": wkt, "wvt": wvt, "wpt": wpt, "bp": bpv,
        })

    res = run_bass_kernel_spmd(
        _get_prog(), in_maps, core_ids=list(range(8)),
        trace=bool(os.environ.get("BASS_TRACE")),
    )
    LAST_RESULT = res

    out = np.empty((B, N, C), np.float32)
    for core in range(8):
        b, qh = core // 2, core % 2
        out[b, qh * NQ:(qh + 1) * NQ, :] = res.results[core]["y"]
    return out


# revision 22
# speedup vs baseline: 1.0038x; 1.0038x over previous
"""Multi-head attention (B=4, N=2048, C=768, H=12) on 8 trn2 NeuronCores.

Sharding: core c handles batch b = c//2 and query rows [ (c%2)*1024, +1024 ).

Per-core engine plan (v2):
  PE     : QKV/proj linears, QK^T (2 heads packed on row groups), PV with the
           two q-chunks packed on column groups (full 128-wide array), ones
           matmuls accumulating softmax denominators on psum rows 0/32/64/96,
           reciprocal-broadcast matmuls.
  ScalarE: exp for even heads (table exp), denominator reciprocal via
           ln -> exp(-x), psum->sbuf drains of the linears.
  VectorE: exp for odd heads via Schraudolph int16 bit-trick (one
           tensor_scalar: i16 = st*A + B, bitcast bf16), attention drains,
           normalize muls.
  DMA    : host-side bf16 inputs; sbuf->sbuf shuffles assemble the normalized
           attention output into projection layout during attention.
  Emission interleaves V (pair 0) and next pair's Q/K (pairs 1-4) into the
  attention loop so the PE never idles and HAM stays at full clock.

PSUM budget (8 banks): st 2x[128,1024] = 4, t 2x[128,512] = 2, den 1, rb 1.
"""

import os
import sys

import numpy as np
import ml_dtypes

sys.path.insert(0, "/opt/trn_rl_repo")

import concourse.bass as bass
from concourse import bacc
import concourse.mybir as mybir
from concourse.tile import TileContext
from concourse.bass_utils import run_bass_kernel_spmd
from concourse.dma_utils import dma_copy

P = 128
C = 768
NK = 2048
NQ = 1024
H = 12
DH = 64
CT = C // P          # 6 c-tiles (contraction tiles for the linears)
KT = NK // P         # 16 key tiles
QCH = 512            # q-chunk (max psum bank free dim for fp32)
SCALE = DH ** -0.5
F32 = mybir.dt.float32
BF16 = mybir.dt.bfloat16
I16 = mybir.dt.int16
EXP = mybir.ActivationFunctionType.Exp
LN = mybir.ActivationFunctionType.Ln

LOG2E = 1.4426950408889634
A16 = 128.0 * LOG2E * SCALE          # fold the 1/8 attention scale
B16 = 127.0 * 128.0 - 0.043677448 * 128.0

LAST_RESULT = None
_PROG = None


def _build_program() -> bass.Bass:
    nc = bacc.Bacc(None, target_bir_lowering=False)

    # host supplies bf16 (halves DMA bytes; matches baseline numerics)
    wqt = nc.dram_tensor("wqt", [C, C], BF16, kind="ExternalInput")
    wkt = nc.dram_tensor("wkt", [C, C], BF16, kind="ExternalInput")
    xq = nc.dram_tensor("xq", [C, NQ], BF16, kind="ExternalInput")
    xt = nc.dram_tensor("xt", [C, NK], BF16, kind="ExternalInput")
    wvt = nc.dram_tensor("wvt", [C, C], BF16, kind="ExternalInput")
    wpt = nc.dram_tensor("wpt", [C, C], BF16, kind="ExternalInput")
    bp = nc.dram_tensor("bp", [1, C], BF16, kind="ExternalInput")
    y = nc.dram_tensor("y", [NQ, C], F32, kind="ExternalOutput")

    with TileContext(nc) as tc:
        with (
            tc.tile_pool(name="persist", bufs=1) as persist,
            tc.tile_pool(name="pt0p", bufs=2) as pt0p,
            tc.tile_pool(name="pt1p", bufs=2) as pt1p,
            tc.tile_pool(name="small", bufs=2) as small,
            tc.tile_pool(name="ysb", bufs=2) as ysb,
            tc.tile_pool(name="ps_st", bufs=2, space="PSUM") as ps_st,
            tc.tile_pool(name="ps_t", bufs=2, space="PSUM") as ps_t,
            tc.tile_pool(name="ps_den", bufs=1, space="PSUM") as ps_den,
            tc.tile_pool(name="ps_rb", bufs=1, space="PSUM") as ps_rb,
        ):
            # ---- load inputs (DMA order = dependency order) ----
            def load(cols, tag):
                return [
                    persist.tile([P, cols], BF16, tag=f"{tag}{i}", name=f"{tag}{i}")
                    for i in range(CT)
                ]

            def dma_tiles(tiles, dram, cols, col0=0):
                for i, t in enumerate(tiles):
                    dma_copy(
                        nc.gpsimd,
                        t[:, col0:col0 + cols],
                        dram[i * P:(i + 1) * P, col0:col0 + cols],
                    )

            wqb = load(C, "wqb")
            wkb = load(C, "wkb")
            xqb = load(NQ, "xqb")
            xtb = load(NK, "xtb")
            wvb = load(C, "wvb")
            wpb = load(C, "wpb")

            def dma_one(t, dram, i, cols, col0=0):
                dma_copy(
                    nc.gpsimd,
                    t[:, col0:col0 + cols],
                    dram[i * P:(i + 1) * P, col0:col0 + cols],
                )

            for i in range(CT):   # Q path first, tile-interleaved
                dma_one(wqb[i], wqt, i, C)
                dma_one(xqb[i], xq, i, NQ)
            for i in range(CT):   # K path
                dma_one(wkb[i], wkt, i, C)
                dma_one(xtb[i], xt, i, NQ, col0=0)
            dma_tiles(wvb, wvt, C)
            dma_tiles(xtb, xt, NQ, col0=NQ)     # second half of keys
            dma_tiles(wpb, wpt, C)

            bpb = persist.tile([1, C], BF16, tag="bpb")
            dma_copy(nc.gpsimd, bpb[:, :], bp[:, :])

            ones = persist.tile([P, P], BF16, tag="ones")
            nc.gpsimd.memset(ones[:, :], 1.0)

            qtb = [persist.tile([P, NQ], BF16, tag=f"qt{i}", name=f"qt{i}") for i in range(CT)]
            ktb = [persist.tile([P, NK], BF16, tag=f"kt{i}", name=f"kt{i}") for i in range(CT)]
            vtb = [persist.tile([P, C], BF16, tag=f"v{i}", name=f"v{i}") for i in range(KT)]
            otb = [persist.tile([P, NQ], BF16, tag=f"ot{i}", name=f"ot{i}") for i in range(CT)]

            def emit_q_chunk(hp, j, pool):
                ps = pool.tile([P, QCH], F32, tag="st")
                for k in range(CT):
                    nc.tensor.matmul(
                        ps[:, :],
                        lhsT=wqb[k][:, hp * P:(hp + 1) * P],
                        rhs=xqb[k][:, j * QCH:(j + 1) * QCH],
                        start=(k == 0), stop=(k == CT - 1),
                    )
                nc.scalar.copy(qtb[hp][:, j * QCH:(j + 1) * QCH], ps[:, :])

            def emit_q(hp, pool):
                for j in range(2):
                    emit_q_chunk(hp, j, pool)

            def emit_k_chunk(hp, j, pool):
                ps = pool.tile([P, QCH], F32, tag="st")
                for k in range(CT):
                    nc.tensor.matmul(
                        ps[:, :],
                        lhsT=wkb[k][:, hp * P:(hp + 1) * P],
                        rhs=xtb[k][:, j * QCH:(j + 1) * QCH],
                        start=(k == 0), stop=(k == CT - 1),
                    )
                nc.scalar.copy(ktb[hp][:, j * QCH:(j + 1) * QCH], ps[:, :])

            def emit_v_chunk(i, ch, pool):
                # vtb[i] = x rows [i*128,+128) @ Wv.T -> [128, 768]
                c0, csz = (0, QCH) if ch == 0 else (QCH, C - QCH)
                ps = pool.tile([P, QCH], F32, tag="st")
                for k in range(CT):
                    nc.tensor.matmul(
                        ps[:, 0:csz],
                        lhsT=xtb[k][:, i * P:(i + 1) * P],
                        rhs=wvb[k][:, c0:c0 + csz],
                        start=(k == 0), stop=(k == CT - 1),
                    )
                nc.scalar.copy(vtb[i][:, c0:c0 + csz], ps[:, 0:csz])

            def emit_v(i, pool):
                for ch in range(2):
                    emit_v_chunk(i, ch, pool)

            # ---- initial linears, ordered by DMA arrival:
            #      wq,xq -> wk,xt-half1 -> wv -> xt-half2 ----
            emit_q(0, ps_st)
            emit_k_chunk(0, 0, ps_st)
            emit_k_chunk(0, 1, ps_st)
            emit_v(0, ps_st)
            emit_q(1, ps_st)
            emit_k_chunk(1, 0, ps_st)
            emit_k_chunk(1, 1, ps_st)
            for i in range(1, 4):
                emit_v(i, ps_st)
            emit_k_chunk(0, 2, ps_st)
            emit_k_chunk(0, 3, ps_st)
            emit_k_chunk(1, 2, ps_st)
            emit_k_chunk(1, 3, ps_st)

            ypart = [
                persist.tile([P, C], F32, tag=f"yp{i}", name=f"yp{i}")
                for i in range(NQ // P)
            ]

            def emit_proj_part(qi, ch):
                # first 4 head-pair contributions + bias for q-tile qi, chunk ch
                c0, csz = (0, QCH) if ch == 0 else (QCH, C - QCH)
                ps = ps_rb.tile([P, QCH], F32, tag="st", name="pp")
                for t in range(4):
                    nc.tensor.matmul(
                        ps[:, 0:csz],
                        lhsT=otb[t][:, qi * P:(qi + 1) * P],
                        rhs=wpb[t][:, c0:c0 + csz],
                        start=(t == 0), stop=False,
                    )
                nc.tensor.matmul(
                    ps[:, 0:csz],
                    lhsT=ones[0:1, 0:P],
                    rhs=bpb[0:1, c0:c0 + csz],
                    start=False, stop=True, tile_position=(0, 0),
                )
                if ch == 0:
                    nc.scalar.copy(ypart[qi][:, c0:c0 + csz], ps[:, 0:csz])
                else:
                    nc.vector.tensor_copy(ypart[qi][:, c0:c0 + csz], ps[:, 0:csz])

            def make_tail_deferred(hp, dsb, tmp0, tmp1):
                # reciprocal (chunked on DVE), broadcast, normalize, assemble —
                # sprinkled into the NEXT pair's k-tile slots
                state = {}

                def recip_chunk(c):
                    if "recb" not in state:
                        state["recb"] = small.tile(
                            [97, QCH], BF16, tag="recb", name="recb"
                        )
                    with nc.allow_low_precision(
                        reason="1/den as bf16 multiplier, matches baseline"
                    ):
                        nc.vector.reciprocal(
                            state["recb"][:, c * DH:(c + 1) * DH],
                            dsb[:, c * DH:(c + 1) * DH],
                        )

                def finish():
                    recb = state["recb"]
                    for h, tmp in ((0, tmp0), (1, tmp1)):
                        rb = ps_rb.tile([P, QCH], F32, tag="st", name="rb")
                        r0, r1 = (0, 32) if h == 0 else (64, 96)
                        nc.tensor.matmul(
                            rb[0:DH, :], lhsT=ones[r0:r0 + 1, 0:DH],
                            rhs=recb[r0:r0 + 1, :],
                            start=True, stop=True, tile_position=(r0, 0),
                        )
                        nc.tensor.matmul(
                            rb[DH:P, :], lhsT=ones[r1:r1 + 1, 0:DH],
                            rhs=recb[r1:r1 + 1, :],
                            start=True, stop=True, tile_position=(r1, 64),
                        )
                        osb = small.tile([P, QCH], BF16, tag="osb", name="osb")
                        nc.vector.tensor_mul(osb[:, :], tmp[:, :], rb[:, :])
                        nc.sync.dma_start(
                            out=otb[hp][h * DH:(h + 1) * DH, 0:QCH],
                            in_=osb[0:DH, :],
                        )
                        nc.sync.dma_start(
                            out=otb[hp][h * DH:(h + 1) * DH, QCH:NQ],
                            in_=osb[DH:P, :],
                        )

                fills = [lambda c=c: recip_chunk(c) for c in range(8)]
                fills.append(finish)
                return fills

            def make_fillers(hp, deferred):
                pe = []
                if hp == 0:
                    for i in range(4, KT):
                        for ch in range(2):
                            pe.append(
                                lambda i=i, ch=ch: emit_v_chunk(i, ch, ps_rb)
                            )
                elif hp < CT - 1:
                    nxt = hp + 1
                    for j in range(2):
                        pe.append(lambda j=j: emit_q_chunk(nxt, j, ps_rb))
                    for j in range(4):
                        pe.append(lambda j=j: emit_k_chunk(nxt, j, ps_rb))
                else:
                    for qi in range(NQ // P):
                        for ch in range(2):
                            pe.append(
                                lambda qi=qi, ch=ch: emit_proj_part(qi, ch)
                            )
                return list(deferred), pe

            deferred = []
            final_tail = None
            for hp in range(CT):
                h0, h1 = 2 * hp, 2 * hp + 1
                dfills, pfills = make_fillers(hp, deferred)
                npf = len(pfills)

                den = ps_den.tile([97, QCH], F32, tag="den")
                t_h0 = ps_t.tile([P, QCH], F32, tag="t")
                t_h1 = ps_t.tile([P, QCH], F32, tag="t")

                def av_den(i, pt0, pt1_i):
                    first, last = (i == 0), (i == KT - 1)
                    v0 = vtb[i][:, h0 * DH:(h0 + 1) * DH]
                    v1 = vtb[i][:, h1 * DH:(h1 + 1) * DH]

                    def pch(h, c):
                        sl = slice(c * QCH, (c + 1) * QCH)
                        if h == 0:
                            return pt0[:, sl]
                        return pt1_i[:, sl].bitcast(BF16)

                    nc.tensor.matmul(
                        t_h0[0:DH, :], lhsT=v0, rhs=pch(0, 0),
                        start=first, stop=last, tile_position=(0, 0),
                    )
                    nc.tensor.matmul(
                        t_h0[DH:P, :], lhsT=v0, rhs=pch(0, 1),
                        start=first, stop=last, tile_position=(0, 64),
                    )
                    nc.tensor.matmul(
                        t_h1[0:DH, :], lhsT=v1, rhs=pch(1, 0),
                        start=first, stop=last, tile_position=(0, 0),
                    )
                    nc.tensor.matmul(
                        t_h1[DH:P, :], lhsT=v1, rhs=pch(1, 1),
                        start=first, stop=last, tile_position=(0, 64),
                    )
                    for pos, (h, c) in zip(
                        (0, 32, 64, 96), ((0, 0), (0, 1), (1, 0), (1, 1))
                    ):
                        nc.tensor.matmul(
                            den[pos:pos + 1, :],
                            lhsT=ones[:, pos:pos + 1],
                            rhs=pch(h, c),
                            start=first, stop=last, tile_position=(0, pos),
                        )

                pending = None   # (i, pt0, pt1_i) awaiting AV + den
                percap = 2 if npf > KT else 1
                for i in range(KT):
                    if i < len(dfills):
                        dfills[i]()
                    for f in range(
                        min(i * percap, npf), min((i + 1) * percap, npf)
                    ):
                        pfills[f]()
                    st1 = ps_st.tile([P, NQ], F32, tag="st")
                    st0 = ps_st.tile([P, NQ], F32, tag="st")
                    for j in range(2):
                        nc.tensor.matmul(
                            st1[:, j * QCH:(j + 1) * QCH],
                            lhsT=ktb[hp][DH:P, i * P:(i + 1) * P],
                            rhs=qtb[hp][DH:P, j * QCH:(j + 1) * QCH],
                            start=True, stop=True,
                            tile_position=(64, 0),
                        )
                        nc.tensor.matmul(
                            st0[:, j * QCH:(j + 1) * QCH],
                            lhsT=ktb[hp][0:DH, i * P:(i + 1) * P],
                            rhs=qtb[hp][0:DH, j * QCH:(j + 1) * QCH],
                            start=True, stop=True,
                            tile_position=(0, 0),
                        )
                    if pending is not None:
                        av_den(*pending)
                    pt1_i = pt1p.tile([P, NQ], I16, tag="pt1")
                    nc.vector.tensor_scalar(
                        pt1_i[:, :], st1[:, :], A16, B16,
                        mybir.AluOpType.mult, mybir.AluOpType.add,
                    )
                    pt0 = pt0p.tile([P, NQ], BF16, tag="pt0")
                    nc.scalar.activation(pt0[:, :], st0[:, :], EXP, scale=SCALE)
                    pending = (i, pt0, pt1_i)
                av_den(*pending)

                # immediate tail: release psum banks quickly (no table loads)
                dsb = small.tile([97, QCH], F32, tag="dsb")
                nc.scalar.copy(dsb[:, :], den[:, :])
                tmp0 = small.tile([P, QCH], F32, tag="tmp0")
                nc.scalar.copy(tmp0[:, :], t_h0[:, :])
                tmp1 = small.tile([P, QCH], F32, tag="tmp1")
                nc.vector.tensor_copy(tmp1[:, :], t_h1[:, :])
                deferred = make_tail_deferred(hp, dsb, tmp0, tmp1)

            # last pair's tail runs right here (nothing left to overlap with)
            for f in deferred:
                f()

            # ---- projection finish: add head-pairs 4,5 to the partials ----
            for qi in range(NQ // P):
                ps = ps_st.tile([P, NQ], F32, tag="st", name="pse")
                for t, last in ((4, False), (5, True)):
                    for (c0, csz) in ((0, QCH), (QCH, C - QCH)):
                        nc.tensor.matmul(
                            ps[:, c0:c0 + csz],
                            lhsT=otb[t][:, qi * P:(qi + 1) * P],
                            rhs=wpb[t][:, c0:c0 + csz],
                            start=(t == 4), stop=last,
                        )
                yt = ysb.tile([P, C], F32, tag="y")
                nc.vector.tensor_add(yt[:, :], ypart[qi][:, :], ps[:, 0:C])
                nc.sync.dma_start(out=y[qi * P:(qi + 1) * P, :], in_=yt[:, :])

    nc.compile()
    return nc


def _get_prog() -> bass.Bass:
    global _PROG
    if _PROG is None:
        _PROG = _build_program()
    return _PROG


def kernel(x, Wq, Wk, Wv, Wp, bp):
    global LAST_RESULT
    bf = ml_dtypes.bfloat16
    x = np.asarray(x, dtype=np.float32)
    wqt = np.ascontiguousarray(np.asarray(Wq, np.float32).T.astype(bf))
    wkt = np.ascontiguousarray(np.asarray(Wk, np.float32).T.astype(bf))
    wvt = np.ascontiguousarray(np.asarray(Wv, np.float32).T.astype(bf))
    wpt = np.ascontiguousarray(np.asarray(Wp, np.float32).T.astype(bf))
    bpv = np.ascontiguousarray(np.asarray(bp, np.float32).reshape(1, C).astype(bf))

    B, N, _ = x.shape
    in_maps = []
    for core in range(8):
        b, qh = core // 2, core % 2
        xtc = np.ascontiguousarray(x[b].T.astype(bf))
        xqc = np.ascontiguousarray(xtc[:, qh * NQ:(qh + 1) * NQ])
        in_maps.append({
            "xt": xtc, "xq": xqc,
            "wqt": wqt, "wkt": wkt, "wvt": wvt, "wpt": wpt, "bp": bpv,
        })

    res = run_bass_kernel_spmd(
        _get_prog(), in_maps, core_ids=list(range(8)),
        trace=bool(os.environ.get("BASS_TRACE")),
    )
    LAST_RESULT = res

    out = np.empty((B, N, C), np.float32)
    for core in range(8):
        b, qh = core // 2, core % 2
        out[b, qh * NQ:(qh + 1) * NQ, :] = res.results[core]["y"]
    return out


# revision 23
# speedup vs baseline: 1.1587x; 1.1544x over previous
"""Multi-head attention (B=4, N=2048, C=768, H=12) on 8 trn2 NeuronCores.

Sharding: core c handles batch b = c//2 and query rows [ (c%2)*1024, +1024 ).

Per-core engine plan (v2):
  PE     : QKV/proj linears, QK^T (2 heads packed on row groups), PV with the
           two q-chunks packed on column groups (full 128-wide array), ones
           matmuls accumulating softmax denominators on psum rows 0/32/64/96,
           reciprocal-broadcast matmuls.
  ScalarE: exp for even heads (table exp), denominator reciprocal via
           ln -> exp(-x), psum->sbuf drains of the linears.
  VectorE: exp for odd heads via Schraudolph int16 bit-trick (one
           tensor_scalar: i16 = st*A + B, bitcast bf16), attention drains,
           normalize muls.
  DMA    : host-side bf16 inputs; sbuf->sbuf shuffles assemble the normalized
           attention output into projection layout during attention.
  Emission interleaves V (pair 0) and next pair's Q/K (pairs 1-4) into the
  attention loop so the PE never idles and HAM stays at full clock.

PSUM budget (8 banks): st 2x[128,1024] = 4, t 2x[128,512] = 2, den 1, rb 1.
"""

import os
import sys

import numpy as np
import ml_dtypes

sys.path.insert(0, "/opt/trn_rl_repo")

import concourse.bass as bass
from concourse import bacc
import concourse.mybir as mybir
from concourse.tile import TileContext
from concourse.bass_utils import run_bass_kernel_spmd
from concourse.dma_utils import dma_copy

P = 128
C = 768
NK = 2048
NQ = 1024
H = 12
DH = 64
CT = C // P          # 6 c-tiles (contraction tiles for the linears)
KT = NK // P         # 16 key tiles
QCH = 512            # q-chunk (max psum bank free dim for fp32)
SCALE = DH ** -0.5
F32 = mybir.dt.float32
BF16 = mybir.dt.bfloat16
I16 = mybir.dt.int16
EXP = mybir.ActivationFunctionType.Exp
LN = mybir.ActivationFunctionType.Ln

LOG2E = 1.4426950408889634
A16 = 128.0 * LOG2E * SCALE          # fold the 1/8 attention scale
B16 = 127.0 * 128.0 - 0.043677448 * 128.0

LAST_RESULT = None
_PROG = None


def _build_program() -> bass.Bass:
    nc = bacc.Bacc(None, target_bir_lowering=False)

    # host supplies bf16 (halves DMA bytes; matches baseline numerics)
    wqt = nc.dram_tensor("wqt", [C, C], BF16, kind="ExternalInput")
    wkt = nc.dram_tensor("wkt", [C, C], BF16, kind="ExternalInput")
    xq = nc.dram_tensor("xq", [C, NQ], BF16, kind="ExternalInput")
    xt = nc.dram_tensor("xt", [C, NK], BF16, kind="ExternalInput")
    wvt = nc.dram_tensor("wvt", [C, C], BF16, kind="ExternalInput")
    wpt = nc.dram_tensor("wpt", [C, C], BF16, kind="ExternalInput")
    bp = nc.dram_tensor("bp", [1, C], BF16, kind="ExternalInput")
    y = nc.dram_tensor("y", [NQ, C], F32, kind="ExternalOutput")

    with TileContext(nc) as tc:
        with (
            tc.tile_pool(name="persist", bufs=1) as persist,
            tc.tile_pool(name="pt0p", bufs=2) as pt0p,
            tc.tile_pool(name="pt1p", bufs=2) as pt1p,
            tc.tile_pool(name="small", bufs=2) as small,
            tc.tile_pool(name="ysb", bufs=2) as ysb,
            tc.tile_pool(name="ps_st", bufs=2, space="PSUM") as ps_st,
            tc.tile_pool(name="ps_t", bufs=2, space="PSUM") as ps_t,
            tc.tile_pool(name="ps_den", bufs=1, space="PSUM") as ps_den,
            tc.tile_pool(name="ps_rb", bufs=1, space="PSUM") as ps_rb,
        ):
            # ---- load inputs (DMA order = dependency order) ----
            def load(cols, tag):
                return [
                    persist.tile([P, cols], BF16, tag=f"{tag}{i}", name=f"{tag}{i}")
                    for i in range(CT)
                ]

            def dma_tiles(tiles, dram, cols, col0=0):
                for i, t in enumerate(tiles):
                    dma_copy(
                        nc.gpsimd,
                        t[:, col0:col0 + cols],
                        dram[i * P:(i + 1) * P, col0:col0 + cols],
                    )

            wqb = load(C, "wqb")
            wkb = load(C, "wkb")
            xqb = load(NQ, "xqb")
            xtb = load(NK, "xtb")
            wvb = load(C, "wvb")
            wpb = load(C, "wpb")

            def dma_one(t, dram, i, cols, col0=0):
                dma_copy(
                    nc.gpsimd,
                    t[:, col0:col0 + cols],
                    dram[i * P:(i + 1) * P, col0:col0 + cols],
                )

            for i in range(CT):   # Q path first, tile-interleaved
                dma_one(wqb[i], wqt, i, C)
                dma_one(xqb[i], xq, i, NQ)
            for i in range(CT):   # K path
                dma_one(wkb[i], wkt, i, C)
                dma_one(xtb[i], xt, i, NQ, col0=0)
            dma_tiles(xtb, xt, NQ, col0=NQ)     # second half of keys
            dma_tiles(wvb, wvt, C)
            dma_tiles(wpb, wpt, C)

            bpb = persist.tile([1, C], BF16, tag="bpb")
            dma_copy(nc.gpsimd, bpb[:, :], bp[:, :])

            ones = persist.tile([P, P], BF16, tag="ones")
            nc.gpsimd.memset(ones[:, :], 1.0)

            qtb = [persist.tile([P, NQ], BF16, tag=f"qt{i}", name=f"qt{i}") for i in range(CT)]
            ktb = [persist.tile([P, NK], BF16, tag=f"kt{i}", name=f"kt{i}") for i in range(CT)]
            vtb = [persist.tile([P, C], BF16, tag=f"v{i}", name=f"v{i}") for i in range(KT)]
            otb = [persist.tile([P, NQ], BF16, tag=f"ot{i}", name=f"ot{i}") for i in range(CT)]

            def emit_q_chunk(hp, j, pool):
                ps = pool.tile([P, QCH], F32, tag="st")
                for k in range(CT):
                    nc.tensor.matmul(
                        ps[:, :],
                        lhsT=wqb[k][:, hp * P:(hp + 1) * P],
                        rhs=xqb[k][:, j * QCH:(j + 1) * QCH],
                        start=(k == 0), stop=(k == CT - 1),
                    )
                nc.scalar.copy(qtb[hp][:, j * QCH:(j + 1) * QCH], ps[:, :])

            def emit_q(hp, pool):
                for j in range(2):
                    emit_q_chunk(hp, j, pool)

            def emit_k_chunk(hp, j, pool):
                ps = pool.tile([P, QCH], F32, tag="st")
                for k in range(CT):
                    nc.tensor.matmul(
                        ps[:, :],
                        lhsT=wkb[k][:, hp * P:(hp + 1) * P],
                        rhs=xtb[k][:, j * QCH:(j + 1) * QCH],
                        start=(k == 0), stop=(k == CT - 1),
                    )
                nc.scalar.copy(ktb[hp][:, j * QCH:(j + 1) * QCH], ps[:, :])

            def emit_v_chunk(i, ch, pool):
                # vtb[i] = x rows [i*128,+128) @ Wv.T -> [128, 768]
                c0, csz = (0, QCH) if ch == 0 else (QCH, C - QCH)
                ps = pool.tile([P, QCH], F32, tag="st")
                for k in range(CT):
                    nc.tensor.matmul(
                        ps[:, 0:csz],
                        lhsT=xtb[k][:, i * P:(i + 1) * P],
                        rhs=wvb[k][:, c0:c0 + csz],
                        start=(k == 0), stop=(k == CT - 1),
                    )
                nc.scalar.copy(vtb[i][:, c0:c0 + csz], ps[:, 0:csz])

            def emit_v(i, pool):
                for ch in range(2):
                    emit_v_chunk(i, ch, pool)

            # ---- initial linears: Q/K pairs 0,1 + V tile 0 (DMA-bound phase) ----
            emit_q(0, ps_st)
            for j in range(4):
                emit_k_chunk(0, j, ps_st)
            emit_v(0, ps_st)
            emit_q(1, ps_st)
            for j in range(4):
                emit_k_chunk(1, j, ps_st)

            ypart = [
                persist.tile([P, C], F32, tag=f"yp{i}", name=f"yp{i}")
                for i in range(NQ // P)
            ]

            def emit_proj_part(qi, ch):
                # first 4 head-pair contributions + bias for q-tile qi, chunk ch
                c0, csz = (0, QCH) if ch == 0 else (QCH, C - QCH)
                ps = ps_rb.tile([P, QCH], F32, tag="st", name="pp")
                for t in range(4):
                    nc.tensor.matmul(
                        ps[:, 0:csz],
                        lhsT=otb[t][:, qi * P:(qi + 1) * P],
                        rhs=wpb[t][:, c0:c0 + csz],
                        start=(t == 0), stop=False,
                    )
                nc.tensor.matmul(
                    ps[:, 0:csz],
                    lhsT=ones[0:1, 0:P],
                    rhs=bpb[0:1, c0:c0 + csz],
                    start=False, stop=True, tile_position=(0, 0),
                )
                if ch == 0:
                    nc.scalar.copy(ypart[qi][:, c0:c0 + csz], ps[:, 0:csz])
                else:
                    nc.vector.tensor_copy(ypart[qi][:, c0:c0 + csz], ps[:, 0:csz])

            def make_tail_deferred(hp, dsb, tmp0, tmp1):
                # reciprocal (chunked on DVE), broadcast, normalize, assemble —
                # sprinkled into the NEXT pair's k-tile slots
                state = {}

                def recip_chunk(c):
                    if "recb" not in state:
                        state["recb"] = small.tile(
                            [97, QCH], BF16, tag="recb", name="recb"
                        )
                    with nc.allow_low_precision(
                        reason="1/den as bf16 multiplier, matches baseline"
                    ):
                        nc.vector.reciprocal(
                            state["recb"][:, c * DH:(c + 1) * DH],
                            dsb[:, c * DH:(c + 1) * DH],
                        )

                def finish():
                    recb = state["recb"]
                    for h, tmp in ((0, tmp0), (1, tmp1)):
                        rb = ps_rb.tile([P, QCH], F32, tag="st", name="rb")
                        r0, r1 = (0, 32) if h == 0 else (64, 96)
                        nc.tensor.matmul(
                            rb[0:DH, :], lhsT=ones[r0:r0 + 1, 0:DH],
                            rhs=recb[r0:r0 + 1, :],
                            start=True, stop=True, tile_position=(r0, 0),
                        )
                        nc.tensor.matmul(
                            rb[DH:P, :], lhsT=ones[r1:r1 + 1, 0:DH],
                            rhs=recb[r1:r1 + 1, :],
                            start=True, stop=True, tile_position=(r1, 64),
                        )
                        osb = small.tile([P, QCH], BF16, tag="osb", name="osb")
                        nc.vector.tensor_mul(osb[:, :], tmp[:, :], rb[:, :])
                        nc.sync.dma_start(
                            out=otb[hp][h * DH:(h + 1) * DH, 0:QCH],
                            in_=osb[0:DH, :],
                        )
                        nc.sync.dma_start(
                            out=otb[hp][h * DH:(h + 1) * DH, QCH:NQ],
                            in_=osb[DH:P, :],
                        )

                fills = [lambda c=c: recip_chunk(c) for c in range(8)]
                fills.append(finish)
                return fills

            def make_fillers(hp, deferred):
                pe = []
                if hp == 0:
                    for i in range(1, KT):
                        for ch in range(2):
                            pe.append(
                                lambda i=i, ch=ch: emit_v_chunk(i, ch, ps_rb)
                            )
                elif hp < CT - 1:
                    nxt = hp + 1
                    for j in range(2):
                        pe.append(lambda j=j: emit_q_chunk(nxt, j, ps_rb))
                    for j in range(4):
                        pe.append(lambda j=j: emit_k_chunk(nxt, j, ps_rb))
                else:
                    for qi in range(NQ // P):
                        for ch in range(2):
                            pe.append(
                                lambda qi=qi, ch=ch: emit_proj_part(qi, ch)
                            )
                return list(deferred), pe

            deferred = []
            final_tail = None
            for hp in range(CT):
                h0, h1 = 2 * hp, 2 * hp + 1
                dfills, pfills = make_fillers(hp, deferred)
                npf = len(pfills)

                den = ps_den.tile([97, QCH], F32, tag="den")
                t_h0 = ps_t.tile([P, QCH], F32, tag="t")
                t_h1 = ps_t.tile([P, QCH], F32, tag="t")

                def av_den(i, pt0, pt1_i):
                    first, last = (i == 0), (i == KT - 1)
                    v0 = vtb[i][:, h0 * DH:(h0 + 1) * DH]
                    v1 = vtb[i][:, h1 * DH:(h1 + 1) * DH]

                    def pch(h, c):
                        sl = slice(c * QCH, (c + 1) * QCH)
                        if h == 0:
                            return pt0[:, sl]
                        return pt1_i[:, sl].bitcast(BF16)

                    nc.tensor.matmul(
                        t_h0[0:DH, :], lhsT=v0, rhs=pch(0, 0),
                        start=first, stop=last, tile_position=(0, 0),
                    )
                    nc.tensor.matmul(
                        t_h0[DH:P, :], lhsT=v0, rhs=pch(0, 1),
                        start=first, stop=last, tile_position=(0, 64),
                    )
                    nc.tensor.matmul(
                        t_h1[0:DH, :], lhsT=v1, rhs=pch(1, 0),
                        start=first, stop=last, tile_position=(0, 0),
                    )
                    nc.tensor.matmul(
                        t_h1[DH:P, :], lhsT=v1, rhs=pch(1, 1),
                        start=first, stop=last, tile_position=(0, 64),
                    )
                    for pos, (h, c) in zip(
                        (0, 32, 64, 96), ((0, 0), (0, 1), (1, 0), (1, 1))
                    ):
                        nc.tensor.matmul(
                            den[pos:pos + 1, :],
                            lhsT=ones[:, pos:pos + 1],
                            rhs=pch(h, c),
                            start=first, stop=last, tile_position=(0, pos),
                        )

                pending = None   # (i, pt0, pt1_i) awaiting AV + den
                percap = 2 if npf > KT else 1
                for i in range(KT):
                    if i < len(dfills):
                        dfills[i]()
                    for f in range(
                        min(i * percap, npf), min((i + 1) * percap, npf)
                    ):
                        pfills[f]()
                    st1 = ps_st.tile([P, NQ], F32, tag="st")
                    st0 = ps_st.tile([P, NQ], F32, tag="st")
                    for j in range(2):
                        nc.tensor.matmul(
                            st1[:, j * QCH:(j + 1) * QCH],
                            lhsT=ktb[hp][DH:P, i * P:(i + 1) * P],
                            rhs=qtb[hp][DH:P, j * QCH:(j + 1) * QCH],
                            start=True, stop=True,
                            tile_position=(64, 0),
                        )
                        nc.tensor.matmul(
                            st0[:, j * QCH:(j + 1) * QCH],
                            lhsT=ktb[hp][0:DH, i * P:(i + 1) * P],
                            rhs=qtb[hp][0:DH, j * QCH:(j + 1) * QCH],
                            start=True, stop=True,
                            tile_position=(0, 0),
                        )
                    if pending is not None:
                        av_den(*pending)
                    pt1_i = pt1p.tile([P, NQ], I16, tag="pt1")
                    nc.vector.tensor_scalar(
                        pt1_i[:, :], st1[:, :], A16, B16,
                        mybir.AluOpType.mult, mybir.AluOpType.add,
                    )
                    pt0 = pt0p.tile([P, NQ], BF16, tag="pt0")
                    nc.scalar.activation(pt0[:, :], st0[:, :], EXP, scale=SCALE)
                    pending = (i, pt0, pt1_i)
                av_den(*pending)

                # immediate tail: release psum banks quickly (no table loads)
                dsb = small.tile([97, QCH], F32, tag="dsb")
                nc.scalar.copy(dsb[:, :], den[:, :])
                tmp0 = small.tile([P, QCH], F32, tag="tmp0")
                nc.scalar.copy(tmp0[:, :], t_h0[:, :])
                tmp1 = small.tile([P, QCH], F32, tag="tmp1")
                nc.vector.tensor_copy(tmp1[:, :], t_h1[:, :])
                deferred = make_tail_deferred(hp, dsb, tmp0, tmp1)

            # last pair's tail runs right here (nothing left to overlap with)
            for f in deferred:
                f()

            # ---- projection finish: add head-pairs 4,5 to the partials ----
            for qi in range(NQ // P):
                ps = ps_st.tile([P, NQ], F32, tag="st", name="pse")
                for t, last in ((4, False), (5, True)):
                    for (c0, csz) in ((0, QCH), (QCH, C - QCH)):
                        nc.tensor.matmul(
                            ps[:, c0:c0 + csz],
                            lhsT=otb[t][:, qi * P:(qi + 1) * P],
                            rhs=wpb[t][:, c0:c0 + csz],
                            start=(t == 4), stop=last,
                        )
                yt = ysb.tile([P, C], F32, tag="y")
                nc.vector.tensor_add(yt[:, :], ypart[qi][:, :], ps[:, 0:C])
                nc.sync.dma_start(out=y[qi * P:(qi + 1) * P, :], in_=yt[:, :])

    nc.compile()
    return nc


def _get_prog() -> bass.Bass:
    global _PROG
    if _PROG is None:
        _PROG = _build_program()
    return _PROG


def kernel(x, Wq, Wk, Wv, Wp, bp):
    global LAST_RESULT
    bf = ml_dtypes.bfloat16
    x = np.asarray(x, dtype=np.float32)
    wqt = np.ascontiguousarray(np.asarray(Wq, np.float32).T.astype(bf))
    wkt = np.ascontiguousarray(np.asarray(Wk, np.float32).T.astype(bf))
    wvt = np.ascontiguousarray(np.asarray(Wv, np.float32).T.astype(bf))
    wpt = np.ascontiguousarray(np.asarray(Wp, np.float32).T.astype(bf))
    bpv = np.ascontiguousarray(np.asarray(bp, np.float32).reshape(1, C).astype(bf))

    B, N, _ = x.shape
    in_maps = []
    for core in range(8):
        b, qh = core // 2, core % 2
        xtc = np.ascontiguousarray(x[b].T.astype(bf))
        xqc = np.ascontiguousarray(xtc[:, qh * NQ:(qh + 1) * NQ])
        in_maps.append({
            "xt": xtc, "xq": xqc,
            "wqt": wqt, "wkt": wkt, "wvt": wvt, "wpt": wpt, "bp": bpv,
        })

    res = run_bass_kernel_spmd(
        _get_prog(), in_maps, core_ids=list(range(8)),
        trace=bool(os.environ.get("BASS_TRACE")),
    )
    LAST_RESULT = res

    out = np.empty((B, N, C), np.float32)
    for core in range(8):
        b, qh = core // 2, core % 2
        out[b, qh * NQ:(qh + 1) * NQ, :] = res.results[core]["y"]
    return out


# revision 24
# speedup vs baseline: 1.1789x; 1.0174x over previous
"""Multi-head attention (B=4, N=2048, C=768, H=12) on 8 trn2 NeuronCores.

Sharding: core c handles batch b = c//2 and query rows [ (c%2)*1024, +1024 ).

Per-core engine plan (v2):
  PE     : QKV/proj linears, QK^T (2 heads packed on row groups), PV with the
           two q-chunks packed on column groups (full 128-wide array), ones
           matmuls accumulating softmax denominators on psum rows 0/32/64/96,
           reciprocal-broadcast matmuls.
  ScalarE: exp for even heads (table exp), denominator reciprocal via
           ln -> exp(-x), psum->sbuf drains of the linears.
  VectorE: exp for odd heads via Schraudolph int16 bit-trick (one
           tensor_scalar: i16 = st*A + B, bitcast bf16), attention drains,
           normalize muls.
  DMA    : host-side bf16 inputs; sbuf->sbuf shuffles assemble the normalized
           attention output into projection layout during attention.
  Emission interleaves V (pair 0) and next pair's Q/K (pairs 1-4) into the
  attention loop so the PE never idles and HAM stays at full clock.

PSUM budget (8 banks): st 2x[128,1024] = 4, t 2x[128,512] = 2, den 1, rb 1.
"""

import os
import sys

import numpy as np
import ml_dtypes

sys.path.insert(0, "/opt/trn_rl_repo")

import concourse.bass as bass
from concourse import bacc
import concourse.mybir as mybir
from concourse.tile import TileContext
from concourse.bass_utils import run_bass_kernel_spmd
from concourse.dma_utils import dma_copy

P = 128
C = 768
NK = 2048
NQ = 1024
H = 12
DH = 64
CT = C // P          # 6 c-tiles (contraction tiles for the linears)
KT = NK // P         # 16 key tiles
QCH = 512            # q-chunk (max psum bank free dim for fp32)
SCALE = DH ** -0.5
F32 = mybir.dt.float32
BF16 = mybir.dt.bfloat16
I16 = mybir.dt.int16
EXP = mybir.ActivationFunctionType.Exp
LN = mybir.ActivationFunctionType.Ln

LOG2E = 1.4426950408889634
A16 = 128.0 * LOG2E * SCALE          # fold the 1/8 attention scale
B16 = 127.0 * 128.0 - 0.043677448 * 128.0

LAST_RESULT = None
_PROG = None


def _build_program() -> bass.Bass:
    nc = bacc.Bacc(None, target_bir_lowering=False)

    # host supplies bf16 (halves DMA bytes; matches baseline numerics)
    wqt = nc.dram_tensor("wqt", [C, C], BF16, kind="ExternalInput")
    wkt = nc.dram_tensor("wkt", [C, C], BF16, kind="ExternalInput")
    xq = nc.dram_tensor("xq", [C, NQ], BF16, kind="ExternalInput")
    xt = nc.dram_tensor("xt", [C, NK], BF16, kind="ExternalInput")
    wvt = nc.dram_tensor("wvt", [C, C], BF16, kind="ExternalInput")
    wpt = nc.dram_tensor("wpt", [C, C], BF16, kind="ExternalInput")
    bp = nc.dram_tensor("bp", [1, C], BF16, kind="ExternalInput")
    y = nc.dram_tensor("y", [NQ, C], F32, kind="ExternalOutput")

    with TileContext(nc) as tc:
        with (
            tc.tile_pool(name="persist", bufs=1) as persist,
            tc.tile_pool(name="pt0p", bufs=2) as pt0p,
            tc.tile_pool(name="pt1p", bufs=2) as pt1p,
            tc.tile_pool(name="small", bufs=2) as small,
            tc.tile_pool(name="ysb", bufs=2) as ysb,
            tc.tile_pool(name="ps_st", bufs=2, space="PSUM") as ps_st,
            tc.tile_pool(name="ps_t", bufs=2, space="PSUM") as ps_t,
            tc.tile_pool(name="ps_den", bufs=1, space="PSUM") as ps_den,
            tc.tile_pool(name="ps_rb", bufs=1, space="PSUM") as ps_rb,
        ):
            # ---- load inputs (DMA order = dependency order) ----
            def load(cols, tag):
                return [
                    persist.tile([P, cols], BF16, tag=f"{tag}{i}", name=f"{tag}{i}")
                    for i in range(CT)
                ]

            def dma_tiles(tiles, dram, cols, col0=0):
                for i, t in enumerate(tiles):
                    dma_copy(
                        nc.gpsimd,
                        t[:, col0:col0 + cols],
                        dram[i * P:(i + 1) * P, col0:col0 + cols],
                    )

            wqb = load(C, "wqb")
            wkb = load(C, "wkb")
            xqb = load(NQ, "xqb")
            xtb = load(NK, "xtb")
            wvb = load(C, "wvb")
            wpb = load(C, "wpb")

            def dma_one(t, dram, i, cols, col0=0):
                dma_copy(
                    nc.gpsimd,
                    t[:, col0:col0 + cols],
                    dram[i * P:(i + 1) * P, col0:col0 + cols],
                )

            for i in range(CT):   # Q path first, tile-interleaved
                dma_one(wqb[i], wqt, i, C)
                dma_one(xqb[i], xq, i, NQ)
            for i in range(CT):   # K path
                dma_one(wkb[i], wkt, i, C)
                dma_one(xtb[i], xt, i, NQ, col0=0)
            dma_tiles(wvb, wvt, C)
            dma_tiles(xtb, xt, NQ, col0=NQ)     # second half of keys
            dma_tiles(wpb, wpt, C)

            bpb = persist.tile([1, C], BF16, tag="bpb")
            dma_copy(nc.gpsimd, bpb[:, :], bp[:, :])

            ones = persist.tile([P, P], BF16, tag="ones")
            nc.gpsimd.memset(ones[:, :], 1.0)

            qtb = [persist.tile([P, NQ], BF16, tag=f"qt{i}", name=f"qt{i}") for i in range(CT)]
            ktb = [persist.tile([P, NK], BF16, tag=f"kt{i}", name=f"kt{i}") for i in range(CT)]
            vtb = [persist.tile([P, C], BF16, tag=f"v{i}", name=f"v{i}") for i in range(KT)]
            otb = [persist.tile([P, NQ], BF16, tag=f"ot{i}", name=f"ot{i}") for i in range(CT)]

            def emit_q_chunk(hp, j, pool):
                ps = pool.tile([P, QCH], F32, tag="st")
                for k in range(CT):
                    nc.tensor.matmul(
                        ps[:, :],
                        lhsT=wqb[k][:, hp * P:(hp + 1) * P],
                        rhs=xqb[k][:, j * QCH:(j + 1) * QCH],
                        start=(k == 0), stop=(k == CT - 1),
                    )
                nc.scalar.copy(qtb[hp][:, j * QCH:(j + 1) * QCH], ps[:, :])

            def emit_q(hp, pool):
                for j in range(2):
                    emit_q_chunk(hp, j, pool)

            def emit_k_chunk(hp, j, pool):
                ps = pool.tile([P, QCH], F32, tag="st")
                for k in range(CT):
                    nc.tensor.matmul(
                        ps[:, :],
                        lhsT=wkb[k][:, hp * P:(hp + 1) * P],
                        rhs=xtb[k][:, j * QCH:(j + 1) * QCH],
                        start=(k == 0), stop=(k == CT - 1),
                    )
                nc.scalar.copy(ktb[hp][:, j * QCH:(j + 1) * QCH], ps[:, :])

            def emit_v_chunk(i, ch, pool):
                # vtb[i] = x rows [i*128,+128) @ Wv.T -> [128, 768]
                c0, csz = (0, QCH) if ch == 0 else (QCH, C - QCH)
                ps = pool.tile([P, QCH], F32, tag="st")
                for k in range(CT):
                    nc.tensor.matmul(
                        ps[:, 0:csz],
                        lhsT=xtb[k][:, i * P:(i + 1) * P],
                        rhs=wvb[k][:, c0:c0 + csz],
                        start=(k == 0), stop=(k == CT - 1),
                    )
                nc.scalar.copy(vtb[i][:, c0:c0 + csz], ps[:, 0:csz])

            def emit_v(i, pool):
                for ch in range(2):
                    emit_v_chunk(i, ch, pool)

            # ---- initial linears, ordered by DMA arrival:
            #      wq,xq -> wk,xt-half1 -> wv -> xt-half2 ----
            emit_q(0, ps_st)
            emit_k_chunk(0, 0, ps_st)
            emit_k_chunk(0, 1, ps_st)
            emit_v(0, ps_st)
            emit_q(1, ps_st)
            emit_k_chunk(1, 0, ps_st)
            emit_k_chunk(1, 1, ps_st)
            for i in range(1, 4):
                emit_v(i, ps_st)
            emit_k_chunk(0, 2, ps_st)
            emit_k_chunk(0, 3, ps_st)
            emit_k_chunk(1, 2, ps_st)
            emit_k_chunk(1, 3, ps_st)

            ypart = [
                persist.tile([P, C], F32, tag=f"yp{i}", name=f"yp{i}")
                for i in range(NQ // P)
            ]

            def emit_proj_part(qi, ch):
                # first 4 head-pair contributions + bias for q-tile qi, chunk ch
                c0, csz = (0, QCH) if ch == 0 else (QCH, C - QCH)
                ps = ps_rb.tile([P, QCH], F32, tag="st", name="pp")
                for t in range(4):
                    nc.tensor.matmul(
                        ps[:, 0:csz],
                        lhsT=otb[t][:, qi * P:(qi + 1) * P],
                        rhs=wpb[t][:, c0:c0 + csz],
                        start=(t == 0), stop=False,
                    )
                nc.tensor.matmul(
                    ps[:, 0:csz],
                    lhsT=ones[0:1, 0:P],
                    rhs=bpb[0:1, c0:c0 + csz],
                    start=False, stop=True, tile_position=(0, 0),
                )
                if ch == 0:
                    nc.scalar.copy(ypart[qi][:, c0:c0 + csz], ps[:, 0:csz])
                else:
                    nc.vector.tensor_copy(ypart[qi][:, c0:c0 + csz], ps[:, 0:csz])

            def make_tail_deferred(hp, dsb, tmp0, tmp1):
                # reciprocal (chunked on DVE), broadcast, normalize, assemble —
                # sprinkled into the NEXT pair's k-tile slots
                state = {}

                def recip_chunk(c):
                    if "recb" not in state:
                        state["recb"] = small.tile(
                            [97, QCH], BF16, tag="recb", name="recb"
                        )
                    with nc.allow_low_precision(
                        reason="1/den as bf16 multiplier, matches baseline"
                    ):
                        nc.vector.reciprocal(
                            state["recb"][:, c * DH:(c + 1) * DH],
                            dsb[:, c * DH:(c + 1) * DH],
                        )

                def finish():
                    recb = state["recb"]
                    for h, tmp in ((0, tmp0), (1, tmp1)):
                        rb = ps_rb.tile([P, QCH], F32, tag="st", name="rb")
                        r0, r1 = (0, 32) if h == 0 else (64, 96)
                        nc.tensor.matmul(
                            rb[0:DH, :], lhsT=ones[r0:r0 + 1, 0:DH],
                            rhs=recb[r0:r0 + 1, :],
                            start=True, stop=True, tile_position=(r0, 0),
                        )
                        nc.tensor.matmul(
                            rb[DH:P, :], lhsT=ones[r1:r1 + 1, 0:DH],
                            rhs=recb[r1:r1 + 1, :],
                            start=True, stop=True, tile_position=(r1, 64),
                        )
                        osb = small.tile([P, QCH], BF16, tag="osb", name="osb")
                        nc.vector.tensor_mul(osb[:, :], tmp[:, :], rb[:, :])
                        nc.sync.dma_start(
                            out=otb[hp][h * DH:(h + 1) * DH, 0:QCH],
                            in_=osb[0:DH, :],
                        )
                        nc.sync.dma_start(
                            out=otb[hp][h * DH:(h + 1) * DH, QCH:NQ],
                            in_=osb[DH:P, :],
                        )

                fills = [lambda c=c: recip_chunk(c) for c in range(8)]
                fills.append(finish)
                return fills

            def make_fillers(hp, deferred):
                pe = []
                if hp == 0:
                    for i in range(4, KT):
                        for ch in range(2):
                            pe.append(
                                lambda i=i, ch=ch: emit_v_chunk(i, ch, ps_rb)
                            )
                elif hp < CT - 1:
                    nxt = hp + 1
                    for j in range(2):
                        pe.append(lambda j=j: emit_q_chunk(nxt, j, ps_rb))
                    for j in range(4):
                        pe.append(lambda j=j: emit_k_chunk(nxt, j, ps_rb))
                else:
                    for qi in range(NQ // P):
                        for ch in range(2):
                            pe.append(
                                lambda qi=qi, ch=ch: emit_proj_part(qi, ch)
                            )
                return list(deferred), pe

            deferred = []
            final_tail = None
            for hp in range(CT):
                h0, h1 = 2 * hp, 2 * hp + 1
                dfills, pfills = make_fillers(hp, deferred)
                npf = len(pfills)

                den = ps_den.tile([97, QCH], F32, tag="den")
                t_h0 = ps_t.tile([P, QCH], F32, tag="t")
                t_h1 = ps_t.tile([P, QCH], F32, tag="t")

                def av_den(i, pt0, pt1_i):
                    first, last = (i == 0), (i == KT - 1)
                    v0 = vtb[i][:, h0 * DH:(h0 + 1) * DH]
                    v1 = vtb[i][:, h1 * DH:(h1 + 1) * DH]

                    def pch(h, c):
                        sl = slice(c * QCH, (c + 1) * QCH)
                        if h == 0:
                            return pt0[:, sl]
                        return pt1_i[:, sl].bitcast(BF16)

                    nc.tensor.matmul(
                        t_h0[0:DH, :], lhsT=v0, rhs=pch(0, 0),
                        start=first, stop=last, tile_position=(0, 0),
                    )
                    nc.tensor.matmul(
                        t_h0[DH:P, :], lhsT=v0, rhs=pch(0, 1),
                        start=first, stop=last, tile_position=(0, 64),
                    )
                    nc.tensor.matmul(
                        t_h1[0:DH, :], lhsT=v1, rhs=pch(1, 0),
                        start=first, stop=last, tile_position=(0, 0),
                    )
                    nc.tensor.matmul(
                        t_h1[DH:P, :], lhsT=v1, rhs=pch(1, 1),
                        start=first, stop=last, tile_position=(0, 64),
                    )
                    for pos, (h, c) in zip(
                        (0, 32, 64, 96), ((0, 0), (0, 1), (1, 0), (1, 1))
                    ):
                        nc.tensor.matmul(
                            den[pos:pos + 1, :],
                            lhsT=ones[:, pos:pos + 1],
                            rhs=pch(h, c),
                            start=first, stop=last, tile_position=(0, pos),
                        )

                pending = None   # (i, pt0, pt1_i) awaiting AV + den
                percap = 2 if npf > KT else 1
                for i in range(KT):
                    if i < len(dfills):
                        dfills[i]()
                    for f in range(
                        min(i * percap, npf), min((i + 1) * percap, npf)
                    ):
                        pfills[f]()
                    st1 = ps_st.tile([P, NQ], F32, tag="st")
                    st0 = ps_st.tile([P, NQ], F32, tag="st")
                    for j in range(2):
                        nc.tensor.matmul(
                            st1[:, j * QCH:(j + 1) * QCH],
                            lhsT=ktb[hp][DH:P, i * P:(i + 1) * P],
                            rhs=qtb[hp][DH:P, j * QCH:(j + 1) * QCH],
                            start=True, stop=True,
                            tile_position=(64, 0),
                        )
                        nc.tensor.matmul(
                            st0[:, j * QCH:(j + 1) * QCH],
                            lhsT=ktb[hp][0:DH, i * P:(i + 1) * P],
                            rhs=qtb[hp][0:DH, j * QCH:(j + 1) * QCH],
                            start=True, stop=True,
                            tile_position=(0, 0),
                        )
                    if pending is not None:
                        av_den(*pending)
                    pt1_i = pt1p.tile([P, NQ], I16, tag="pt1")
                    nc.vector.tensor_scalar(
                        pt1_i[:, :], st1[:, :], A16, B16,
                        mybir.AluOpType.mult, mybir.AluOpType.add,
                    )
                    pt0 = pt0p.tile([P, NQ], BF16, tag="pt0")
                    nc.scalar.activation(pt0[:, :], st0[:, :], EXP, scale=SCALE)
                    pending = (i, pt0, pt1_i)
                av_den(*pending)

                # immediate tail: release psum banks quickly (no table loads)
                dsb = small.tile([97, QCH], F32, tag="dsb")
                nc.scalar.copy(dsb[:, :], den[:, :])
                tmp0 = small.tile([P, QCH], F32, tag="tmp0")
                nc.scalar.copy(tmp0[:, :], t_h0[:, :])
                tmp1 = small.tile([P, QCH], F32, tag="tmp1")
                nc.vector.tensor_copy(tmp1[:, :], t_h1[:, :])
                deferred = make_tail_deferred(hp, dsb, tmp0, tmp1)

            # last pair's tail runs right here (nothing left to overlap with)
            for f in deferred:
                f()

            # ---- projection finish: add head-pairs 4,5 to the partials ----
            for qi in range(NQ // P):
                ps = ps_st.tile([P, NQ], F32, tag="st", name="pse")
                for t, last in ((4, False), (5, True)):
                    for (c0, csz) in ((0, QCH), (QCH, C - QCH)):
                        nc.tensor.matmul(
                            ps[:, c0:c0 + csz],
                            lhsT=otb[t][:, qi * P:(qi + 1) * P],
                            rhs=wpb[t][:, c0:c0 + csz],
                            start=(t == 4), stop=last,
                        )
                yt = ysb.tile([P, C], F32, tag="y")
                nc.vector.tensor_add(yt[:, :], ypart[qi][:, :], ps[:, 0:C])
                nc.sync.dma_start(out=y[qi * P:(qi + 1) * P, :], in_=yt[:, :])

    nc.compile()
    return nc


def _get_prog() -> bass.Bass:
    global _PROG
    if _PROG is None:
        _PROG = _build_program()
    return _PROG


def kernel(x, Wq, Wk, Wv, Wp, bp):
    global LAST_RESULT
    bf = ml_dtypes.bfloat16
    x = np.asarray(x, dtype=np.float32)
    wqt = np.ascontiguousarray(np.asarray(Wq, np.float32).T.astype(bf))
    wkt = np.ascontiguousarray(np.asarray(Wk, np.float32).T.astype(bf))
    wvt = np.ascontiguousarray(np.asarray(Wv, np.float32).T.astype(bf))
    wpt = np.ascontiguousarray(np.asarray(Wp, np.float32).T.astype(bf))
    bpv = np.ascontiguousarray(np.asarray(bp, np.float32).reshape(1, C).astype(bf))

    B, N, _ = x.shape
    in_maps = []
    for core in range(8):
        b, qh = core // 2, core % 2
        xtc = np.ascontiguousarray(x[b].T.astype(bf))
        xqc = np.ascontiguousarray(xtc[:, qh * NQ:(qh + 1) * NQ])
        in_maps.append({
            "xt": xtc, "xq": xqc,
            "wqt": wqt, "wkt": wkt, "wvt": wvt, "wpt": wpt, "bp": bpv,
        })

    res = run_bass_kernel_spmd(
        _get_prog(), in_maps, core_ids=list(range(8)),
        trace=bool(os.environ.get("BASS_TRACE")),
    )
    LAST_RESULT = res

    out = np.empty((B, N, C), np.float32)
    for core in range(8):
        b, qh = core // 2, core % 2
        out[b, qh * NQ:(qh + 1) * NQ, :] = res.results[core]["y"]
    return out
